# revision 1
# baseline (speedup 1.0000x reference)
"""Bahdanau (additive) attention kernel for Trainium2, 8 NeuronCores.

Full-input contract: kernel(**inputs) takes the unsharded numpy inputs and
returns the full [TQ, B, D] output. Internally shards (batch, query-half)
across 8 cores (B=4 x 2 halves of Tq), runs a Bass/Tile kernel per core via
run_bass_kernel_spmd, and reassembles.

Sparsity: masked value positions contribute exactly 0 to the softmax
(score + -1e9 -> exp underflows to 0), so the host gathers only the valid
value positions per batch (mask is input data), pads to a common TVE
(multiple of 8), and the device program is compiled for that TVE (cached).

Per-core program (b = batch, 128 local queries, TVE gathered positions):
  warmup matmuls flip the PE clock gate (HAM) during the input DMAs
  wqT[u,q] = sum_d W1[d,u] q[q,d]          (PE matmul, fp32 -> SBUF)
  wkT[u,v] = sum_d W2[d,u] v[v,d]          (PE matmul, fp32; stays in PSUM)
  g_q[u,v] = tanh(wkT[u,v] + wqT[u,q])     (ACT, per-partition bias = wqT[:,q])
  scores[q,v] = mka[v] + sum_u scale[u] g_q[u,v]
      (PE: K=1 bf16 mask matmul opens the PSUM accumulation, then per q a
       float32r matmul with sliding-window lhsT = sigma in column q, else 0)
  e = exp(scores) in bank slices, each with fused row-sum (ACT accum_out)
  ctx[q,d] = (1/ssum[q]) sum_v e[q,v] v[v,d]  (PE transpose + matmuls,
       chunk-pipelined with the exp slices; DVE scale on the output copy)
"""

import sys

if "/opt/trn_rl_repo" not in sys.path:
    sys.path.insert(0, "/opt/trn_rl_repo")

import numpy as np

TQ, TV, B, D, U = 256, 1024, 4, 128, 128
NCORES = 8
TQL = 128  # local queries per core (Tq=256 split in 2 per batch)
NEG_INF = -1e9

# Score-contraction matmul dtype: "f32r" (reduced fp32, ~1e-4 rel err) or
# "bf16" (~1.5e-3) or "f32" (exact but 4 cyc/row).
SCORE_DT = "f32r"

_CACHE = {}


def _bank_pieces(tve):
    """Split [0, tve) into PSUM-bank-aligned matmul slices (<=512 each)."""
    pieces = []
    a = 0
    while a < tve:
        n = min(512, tve - a)
        pieces.append((a, n))
        a += n
    return pieces


def _build_nc(tve):
    import concourse.bacc as bacc
    import concourse.mybir as mybir
    import concourse.tile as tile
    from contextlib import ExitStack

    f32 = mybir.dt.float32
    f32r = mybir.dt.float32r
    bf16 = mybir.dt.bfloat16
    AFT = mybir.ActivationFunctionType

    nc = bacc.Bacc("TRN2", target_bir_lowering=False, debug=False,
                   num_devices=NCORES)

    sdt = {"f32r": f32r, "bf16": bf16, "f32": f32}[SCORE_DT]

    NVC = -(-tve // 128)              # ctx chunks (last may be partial)
    pieces = _bank_pieces(tve)

    wpack = nc.dram_tensor("wpack", [D, 3 * 128], f32,
                           kind="ExternalInput").ap()
    vt = nc.dram_tensor("vt", [D, tve], f32, kind="ExternalInput").ap()
    vnp = nc.dram_tensor("vnp", [128, NVC * D], f32,
                         kind="ExternalInput").ap()
    sige = nc.dram_tensor("sige", [U, 2 * TQL - 1], sdt,
                          kind="ExternalInput").ap()
    mpack = nc.dram_tensor("mpack", [1, tve + TQL], bf16,
                           kind="ExternalInput").ap()
    ident = nc.dram_tensor("ident", [128, 128], f32, kind="ExternalInput").ap()
    out = nc.dram_tensor("out", [TQL, D], f32, kind="ExternalOutput").ap()

    with tile.TileContext(nc) as tc:
        with ExitStack() as ctx:
            consts = ctx.enter_context(tc.tile_pool(name="consts", bufs=1))
            gpool = ctx.enter_context(tc.tile_pool(name="g", bufs=3))
            smp = ctx.enter_context(tc.tile_pool(name="sm", bufs=1))
            etp = ctx.enter_context(tc.tile_pool(name="et", bufs=2))
            ps1 = ctx.enter_context(tc.tile_pool(name="ps1", bufs=1,
                                                 space="PSUM"))
            pst = ctx.enter_context(tc.tile_pool(name="pst", bufs=2,
                                                 space="PSUM"))

            wpack_sb = consts.tile([D, 3 * 128], f32, tag="wpack")
            w1_sb = wpack_sb[:, 0:128]
            qt_sb = wpack_sb[:, 128:256]
            w2_sb = wpack_sb[:, 256:384]
            vt_sb = consts.tile([D, tve], f32, tag="vt")
            vnp_sb = consts.tile([128, NVC * D], f32, tag="vnp")
            sig_sb = consts.tile([U, 2 * TQL - 1], sdt, tag="sig")
            mpack_sb = consts.tile([1, tve + TQL], bf16, tag="mpack")
            mka_sb = mpack_sb[:, 0:tve]
            ones_sb = mpack_sb[:, tve:tve + TQL]
            id_sb = consts.tile([128, 128], f32, tag="id")
            wqT_sb = consts.tile([U, TQL], f32, tag="wqT")

            # preload the exp/tanh ACT table set during the input DMAs
            warm_in = consts.tile([128, 1], f32, tag="warm_in")
            warm_out = consts.tile([128, 1], f32, tag="warm_out")
            nc.gpsimd.memset(warm_in[:], 0.0)
            nc.scalar.activation(warm_out[:], warm_in[:], AFT.Tanh)

            # flip the PE clock gate (HAM) warm with ~3.4us of dummy
            # matmuls while the input DMAs land + complete
            warm_mm = consts.tile([128, 512], bf16, tag="warm_mm")
            nc.gpsimd.memset(warm_mm[:], 0.0)
            warm_ps = pst.tile([128, 512], f32, tag="tp")
            for _ in range(6):
                nc.tensor.matmul(warm_ps[:], lhsT=warm_mm[:, 0:128],
                                 rhs=warm_mm[:], start=True, stop=True)

            # one serialized DMA queue, priority order: the 16 SDMA engines
            # are shared, so concurrent bulk DMAs would delay vt's completion
            # receipt (which gates wk -> first tanh)
            nc.sync.dma_start(vt_sb[:], vt[:])
            nc.sync.dma_start(wpack_sb[:], wpack[:])
            nc.scalar.dma_start(sig_sb[:], sige[:])
            nc.sync.dma_start(mpack_sb[:], mpack[:])
            nc.sync.dma_start(id_sb[:], ident[:])
            nc.sync.dma_start(vnp_sb[:], vnp[:])

            # wkT = W2.T @ vT -> SBUF (ACT reads SBUF faster than PSUM)
            wk_ps = ps1.tile([U, tve], f32, tag="wk")
            wk_sb = consts.tile([U, tve], f32, tag="wk_sb")
            for a, n in pieces:
                nc.tensor.matmul(wk_ps[:, a:a + n], lhsT=w2_sb[:],
                                 rhs=vt_sb[:, a:a + n])
            nc.scalar.copy(wk_sb[:], wk_ps[:])

            # wqT = W1.T @ qT  -> copy to SBUF (ACT bias source)
            wq_ps = ps1.tile([U, TQL], f32, tag="wq")
            nc.tensor.matmul(wq_ps[:], lhsT=w1_sb[:], rhs=qt_sb[:])
            nc.vector.tensor_copy(wqT_sb[:], wq_ps[:])

            scores_ps = ps1.tile([TQL, tve], f32, tag="scores")
            # pad/mask add opens+closes the accumulation-group bookkeeping:
            # scores[m, v] = mka[v]; later matmuls accumulate per-element.
            for a, n in pieces:
                nc.tensor.matmul(scores_ps[:, a:a + n],
                                 lhsT=ones_sb[:], rhs=mka_sb[:, a:a + n],
                                 start=True, stop=True)
            # largest pack (fewest ACT slot semaphores) that keeps the
            # 3-buffered g pool within the SBUF budget
            QPACK = 16
            while QPACK > 1 and QPACK * tve * 4 * 3 > 110_000:
                QPACK //= 2
            for q0 in range(0, TQL, QPACK):
                g2 = gpool.tile([U, QPACK, tve], sdt, tag="g")
                for i in range(QPACK):
                    q = q0 + i
                    nc.scalar.activation(g2[:, i, :], wk_sb[:], AFT.Tanh,
                                         bias=wqT_sb[:, q:q + 1])
                    lw = sig_sb[:, TQL - 1 - q: 2 * TQL - 1 - q]
                    for a, n in pieces:
                        nc.tensor.matmul(scores_ps[:, a:a + n],
                                         lhsT=lw, rhs=g2[:, i, a:a + n],
                                         start=False, stop=False,
                                         skip_group_check=True)

            # exp in bank slices with fused row-sums; ctx chunks pipeline in
            exp_sb = smp.tile([TQL, tve], f32, tag="exp")
            ssums = smp.tile([TQL, len(pieces)], f32, tag="ssums")
            ssum = smp.tile([TQL, 1], f32, tag="ssum")
            rins = smp.tile([TQL, 1], f32, tag="rins")
            for j, (a, n) in enumerate(pieces):
                nc.scalar.activation(exp_sb[:, a:a + n], scores_ps[:, a:a + n],
                                     AFT.Exp, accum_out=ssums[:, j:j + 1])
            if len(pieces) > 1:
                nc.vector.reduce_sum(ssum[:], ssums[:],
                                     axis=mybir.AxisListType.X)
            else:
                nc.vector.tensor_copy(ssum[:], ssums[:])
            nc.vector.reciprocal(rins[:], ssum[:])

            # ctx = softmax @ v  (transpose exp chunks, accumulate matmuls)
            ctx_ps = ps1.tile([TQL, D], f32, tag="ctx")
            for k in range(NVC):
                n = min(128, tve - k * 128)
                tp = pst.tile([128, 128], f32, tag="tp")
                nc.tensor.transpose(tp[:n, :],
                                    exp_sb[:, k * 128:k * 128 + n], id_sb[:])
                et = etp.tile([128, 128], f32, tag="et")
                if k % 2 == 0:
                    nc.vector.tensor_copy(et[:n, :], tp[:n, :])
                else:
                    nc.scalar.copy(et[:n, :], tp[:n, :])
                nc.tensor.matmul(ctx_ps[:], lhsT=et[:n, :],
                                 rhs=vnp_sb[:n, k * D:(k + 1) * D],
                                 start=(k == 0), stop=(k == NVC - 1))

            out_sb = smp.tile([TQL, D], f32, tag="out")
            nc.vector.tensor_scalar_mul(out_sb[:], ctx_ps[:], rins[:])
            nc.sync.dma_start(out[:], out_sb[:])

    nc.compile()
    return nc


def get_nc(tve=TV):
    key = ("nc", tve)
    if key not in _CACHE:
        _CACHE[key] = _build_nc(tve)
    return _CACHE[key]


def prep_in_maps(query, value, mask, W1, W2, scale):
    """Gather valid value positions per batch; returns (in_maps, tve)."""
    import ml_dtypes

    query = np.asarray(query, dtype=np.float32)
    value = np.asarray(value, dtype=np.float32)
    mask = np.asarray(mask)
    W1 = np.ascontiguousarray(np.asarray(W1, dtype=np.float32))
    W2 = np.ascontiguousarray(np.asarray(W2, dtype=np.float32))
    scale = np.asarray(scale, dtype=np.float32)

    idxs = [np.nonzero(mask[:, b])[0] for b in range(B)]
    nv_max = max(1, max(len(ix) for ix in idxs))
    tve = min(TV, -(-nv_max // 4) * 4)
    NVC = -(-tve // 128)

    bf16_np = np.dtype(ml_dtypes.bfloat16)
    sdt_np = bf16_np if SCORE_DT == "bf16" else np.float32
    sige = np.zeros((U, 2 * TQL - 1), sdt_np)
    sige[:, TQL - 1] = scale.astype(sdt_np)
    ident = np.eye(128, dtype=np.float32)
    ones1 = np.ones((1, TQL), bf16_np)

    in_maps = []
    for c in range(NCORES):
        b, q0 = c // 2, (c % 2) * TQL
        ix = idxs[b]
        nv = len(ix)
        vg = np.zeros((NVC * 128, D), np.float32)
        vg[:nv] = value[ix, b, :]
        mka = np.zeros((1, tve), bf16_np)
        mka[0, nv:] = NEG_INF
        wpack = np.concatenate(
            [W1, np.ascontiguousarray(query[q0:q0 + TQL, b, :].T), W2],
            axis=1)
        mpack = np.concatenate([mka, ones1], axis=1)
        in_maps.append({
            "wpack": np.ascontiguousarray(wpack),
            "vt": np.ascontiguousarray(vg[:tve].T),
            "vnp": np.ascontiguousarray(
                vg.reshape(NVC, 128, D).transpose(1, 0, 2)
                .reshape(128, NVC * D)),
            "sige": sige,
            "mpack": np.ascontiguousarray(mpack),
            "ident": ident,
        })
    return in_maps, tve


def run(query, value, mask, W1, W2, scale, trace=False):
    from concourse.bass_utils import run_bass_kernel_spmd

    in_maps, tve = prep_in_maps(query, value, mask, W1, W2, scale)
    nc = get_nc(tve)
    res = run_bass_kernel_spmd(nc, in_maps, list(range(NCORES)), trace=trace)
    out = np.empty((TQ, B, D), np.float32)
    for c in range(NCORES):
        b, q0 = c // 2, (c % 2) * TQL
        out[q0:q0 + TQL, b, :] = res.results[c]["out"]
    return out, res


def kernel(query, value, mask, W1, W2, scale):
    out, _ = run(query, value, mask, W1, W2, scale, trace=False)
    return out



# revision 3
# speedup vs baseline: 4.1930x; 4.1930x over previous
"""Bahdanau (additive) attention kernel for Trainium2, 8 NeuronCores.

Full-input contract: kernel(**inputs) takes the unsharded numpy inputs and
returns the full [TQ, B, D] output. Internally shards (batch, query-half)
across 8 cores (B=4 x 2 halves of Tq), runs a Bass/Tile kernel per core via
run_bass_kernel_spmd, and reassembles.

Sparsity: masked value positions contribute exactly 0 to the softmax
(score + -1e9 -> exp underflows to 0), so the host gathers only the valid
value positions per batch (mask is input data), pads to a common TVE
(multiple of 8), and the device program is compiled for that TVE (cached).

Rank-K score decomposition: the additive score
  scores[q,v] = sum_u s_u tanh(a[q,u] + b[v,u]),  a = qW1, b = vW2
is evaluated via a shifted-tanh basis expansion of the bivariate function
  tanh(a+b) ~= f0(a) + f1(a)*b + sum_k fk(a) tanh(b + t_k)
whose per-a coefficients are solved on the host by Gaussian-weighted least
squares (a = wq is host-computed; it is O(Tq*D*U), tiny next to the
O(Tq*Tv*U) score tensor). Folding s_u into the coefficients gives
  scores[q,v] = h0[q] + sum_m H_m[:,q] . TB_m[:,v]
so the device only computes K+1 activation passes over [U, TVE] (the basis
tiles TB_m) and K+2 PE matmuls - instead of Tq tanh passes. h0 is applied
as the per-partition bias of the softmax exp activation (free).

Per-core program (b = batch, 128 local queries, TVE gathered positions):
  warmup matmuls flip the PE clock gate (HAM) during the input DMAs
  wk[u,v] = sum_d W2[d,u] v[v,d]           (PE matmul, bf16 -> PSUM)
  TB_k[u,v] = tanh(wk[u,v] + t_k)          (ACT reads PSUM, bf16 out)
  scores[q,v] = mka[v] + H_1 . wk + sum_k H_k . TB_k   (PE, bf16 PSUM accum)
  e = exp(scores + h0[q]) bank slices, fused row-sum (ACT accum_out)
  ctx[q,d] = (1/ssum[q]) sum_v e[q,v] v[v,d]  (PE transpose + bf16 matmuls;
       DVE scale on the output copy)
"""

import sys

if "/opt/trn_rl_repo" not in sys.path:
    sys.path.insert(0, "/opt/trn_rl_repo")

import numpy as np

TQ, TV, B, D, U = 256, 1024, 4, 128, 128
NCORES = 8
TQL = 128  # local queries per core (Tq=256 split in 2 per batch)
NEG_INF = -1e9

# Basis config: K tanh shifts spread over the host-side a = qW1 range.
KB = 12
SHIFT_MAX = 4.8
AGRID_N = 2001
AGRID_MAX = 5.4
BGRID_N = 601
BGRID_MAX = 6.0

_CACHE = {}


def _bank_pieces(tve):
    """Split [0, tve) into PSUM-bank-aligned matmul slices (<=512 each)."""
    pieces = []
    a = 0
    while a < tve:
        n = min(512, tve - a)
        pieces.append((a, n))
        a += n
    return pieces


def _basis_tables():
    """Least-squares coefficient tables for the shifted-tanh expansion.

    Returns (t, agrid, Fg) with Fg[i, m] the coefficient of basis m
    (m=0 const, m=1 identity, m=2.. tanh(b+t_{m-2})) for a = agrid[i]:
      tanh(a + b) ~= sum_m Fg[i, m] * phi_m(b)   (b ~ N(0,1)-weighted)
    """
    key = "basis"
    if key in _CACHE:
        return _CACHE[key]
    t = np.linspace(-SHIFT_MAX, SHIFT_MAX, KB)
    bg = np.linspace(-BGRID_MAX, BGRID_MAX, BGRID_N)
    sw = np.sqrt(np.exp(-bg ** 2 / 4))
    cols = [np.ones_like(bg), bg] + [np.tanh(bg + tk) for tk in t]
    A = (np.vstack(cols) * sw).T                  # [nb, M]
    P = np.linalg.pinv(A, rcond=1e-12)            # [M, nb]
    agrid = np.linspace(-AGRID_MAX, AGRID_MAX, AGRID_N)
    Y = np.tanh(agrid[:, None] + bg[None, :]) * sw  # [na, nb]
    Fg = Y @ P.T                                  # [na, M]
    _CACHE[key] = (t, agrid, Fg.astype(np.float64))
    return _CACHE[key]


def _build_nc(tve):
    import concourse.bacc as bacc
    import concourse.mybir as mybir
    import concourse.tile as tile
    from contextlib import ExitStack

    f32 = mybir.dt.float32
    bf16 = mybir.dt.bfloat16
    AFT = mybir.ActivationFunctionType

    nc = bacc.Bacc("TRN2", target_bir_lowering=False, debug=False,
                   num_devices=NCORES)

    NVC = -(-tve // 128)              # ctx chunks (last may be partial)
    pieces = _bank_pieces(tve)
    M = KB + 2                        # const (exp bias) + identity + K tanh

    vt = nc.dram_tensor("vt", [D, tve], bf16, kind="ExternalInput").ap()
    w2 = nc.dram_tensor("w2", [D, U], bf16, kind="ExternalInput").ap()
    smalls = nc.dram_tensor("smalls", [128, KB + 1], f32,
                            kind="ExternalInput").ap()
    mpack = nc.dram_tensor("mpack", [1, tve + TQL], bf16,
                           kind="ExternalInput").ap()
    hpack = nc.dram_tensor("hpack", [U, (M - 1) * TQL], bf16,
                           kind="ExternalInput").ap()
    vnp = nc.dram_tensor("vnp", [128, NVC * D], bf16,
                         kind="ExternalInput").ap()
    ident = nc.dram_tensor("ident", [128, 128], bf16,
                           kind="ExternalInput").ap()
    out = nc.dram_tensor("out", [TQL, D], f32, kind="ExternalOutput").ap()

    with tile.TileContext(nc) as tc:
        with ExitStack() as ctx:
            consts = ctx.enter_context(tc.tile_pool(name="consts", bufs=1))
            tbp = ctx.enter_context(tc.tile_pool(name="tb", bufs=3))
            smp = ctx.enter_context(tc.tile_pool(name="sm", bufs=1))
            etp = ctx.enter_context(tc.tile_pool(name="et", bufs=2))
            ps1 = ctx.enter_context(tc.tile_pool(name="ps1", bufs=1,
                                                 space="PSUM"))
            pst = ctx.enter_context(tc.tile_pool(name="pst", bufs=2,
                                                 space="PSUM"))

            vt_sb = consts.tile([D, tve], bf16, tag="vt")
            w2_sb = consts.tile([D, U], bf16, tag="w2")
            smalls_sb = consts.tile([128, KB + 1], f32, tag="smalls")
            mpack_sb = consts.tile([1, tve + TQL], bf16, tag="mpack")
            mka_sb = mpack_sb[:, 0:tve]
            ones_sb = mpack_sb[:, tve:tve + TQL]
            hpack_sb = consts.tile([U, (M - 1) * TQL], bf16, tag="hpack")
            vnp_sb = consts.tile([128, NVC * D], bf16, tag="vnp")
            id_sb = consts.tile([128, 128], bf16, tag="id")

            # preload the exp/tanh ACT table set during the input DMAs
            warm_in = consts.tile([128, 1], f32, tag="warm_in")
            warm_out = consts.tile([128, 1], f32, tag="warm_out")
            nc.gpsimd.memset(warm_in[:], 0.0)
            nc.scalar.activation(warm_out[:], warm_in[:], AFT.Tanh)

            # flip the PE clock gate (HAM) warm with dummy matmuls while
            # the input DMAs land + complete
            warm_mm = consts.tile([128, 512], bf16, tag="warm_mm")
            nc.gpsimd.memset(warm_mm[:], 0.0)
            warm_ps = pst.tile([128, 512], f32, tag="tp")
            for _ in range(6):
                nc.tensor.matmul(warm_ps[:], lhsT=warm_mm[:, 0:128],
                                 rhs=warm_mm[:], start=True, stop=True)

            # one serialized DMA queue, priority order: vt gates wk -> the
            # whole tanh-basis pipeline; hpack slices are consumed in order
            nc.sync.dma_start(vt_sb[:], vt[:])
            nc.sync.dma_start(w2_sb[:], w2[:])
            nc.sync.dma_start(smalls_sb[:], smalls[:])
            nc.sync.dma_start(mpack_sb[:], mpack[:])
            nc.sync.dma_start(hpack_sb[:], hpack[:])
            nc.scalar.dma_start(vnp_sb[:], vnp[:])
            nc.scalar.dma_start(id_sb[:], ident[:])

            # wk = W2.T @ vT -> PSUM (ACT reads PSUM at lower latency)
            wk_ps = ps1.tile([U, tve], f32, tag="wk")
            for a, n in pieces:
                nc.tensor.matmul(wk_ps[:, a:a + n], lhsT=w2_sb[:],
                                 rhs=vt_sb[:, a:a + n])
            # bf16 copy for the identity-basis matmul rhs (off ACT's path)
            wkb_sb = consts.tile([U, tve], bf16, tag="wkb")
            nc.vector.tensor_copy(wkb_sb[:], wk_ps[:])

            scores_ps = ps1.tile([TQL, tve], f32, tag="scores")
            # pad/mask add opens the accumulation-group bookkeeping:
            # scores[q, v] = mka[v]; later matmuls accumulate per-element.
            for a, n in pieces:
                nc.tensor.matmul(scores_ps[:, a:a + n],
                                 lhsT=ones_sb[:], rhs=mka_sb[:, a:a + n],
                                 start=True, stop=True)
            # identity-basis term: scores += H_1 . wk
            for a, n in pieces:
                nc.tensor.matmul(scores_ps[:, a:a + n],
                                 lhsT=hpack_sb[:, 0:TQL],
                                 rhs=wkb_sb[:, a:a + n],
                                 start=False, stop=False,
                                 skip_group_check=True)
            # K shifted-tanh basis tiles; PE consumes each as ACT emits it
            for k in range(KB):
                tb = tbp.tile([U, tve], bf16, tag="tb")
                nc.scalar.activation(tb[:], wk_ps[:], AFT.Tanh,
                                     bias=smalls_sb[:, k:k + 1])
                lw = hpack_sb[:, (k + 1) * TQL:(k + 2) * TQL]
                for a, n in pieces:
                    nc.tensor.matmul(scores_ps[:, a:a + n],
                                     lhsT=lw, rhs=tb[:, a:a + n],
                                     start=False, stop=False,
                                     skip_group_check=True)

            # exp in bank slices with fused row-sums; the const basis term
            # h0[q] rides along as the per-partition activation bias
            exp_sb = smp.tile([TQL, tve], bf16, tag="exp")
            ssums = smp.tile([TQL, len(pieces)], f32, tag="ssums")
            ssum = smp.tile([TQL, 1], f32, tag="ssum")
            rins = smp.tile([TQL, 1], f32, tag="rins")
            for j, (a, n) in enumerate(pieces):
                nc.scalar.activation(exp_sb[:, a:a + n], scores_ps[:, a:a + n],
                                     AFT.Exp, bias=smalls_sb[:, KB:KB + 1],
                                     accum_out=ssums[:, j:j + 1])
            if len(pieces) > 1:
                nc.vector.reduce_sum(ssum[:], ssums[:],
                                     axis=mybir.AxisListType.X)
            else:
                nc.vector.tensor_copy(ssum[:], ssums[:])
            nc.vector.reciprocal(rins[:], ssum[:])

            # ctx = softmax @ v  (transpose exp chunks, accumulate matmuls)
            ctx_ps = ps1.tile([TQL, D], f32, tag="ctx")
            for k in range(NVC):
                n = min(128, tve - k * 128)
                tp = pst.tile([128, 128], bf16, tag="tp")
                nc.tensor.transpose(tp[:n, :],
                                    exp_sb[:, k * 128:k * 128 + n], id_sb[:])
                et = etp.tile([128, 128], bf16, tag="et")
                if k % 2 == 0:
                    nc.vector.tensor_copy(et[:n, :], tp[:n, :])
                else:
                    nc.scalar.copy(et[:n, :], tp[:n, :])
                nc.tensor.matmul(ctx_ps[:], lhsT=et[:n, :],
                                 rhs=vnp_sb[:n, k * D:(k + 1) * D],
                                 start=(k == 0), stop=(k == NVC - 1))

            out_sb = smp.tile([TQL, D], f32, tag="out")
            nc.vector.tensor_scalar_mul(out_sb[:], ctx_ps[:], rins[:])
            nc.sync.dma_start(out[:], out_sb[:])

    nc.compile()
    return nc


def get_nc(tve=TV):
    key = ("nc", tve)
    if key not in _CACHE:
        _CACHE[key] = _build_nc(tve)
    return _CACHE[key]


def prep_in_maps(query, value, mask, W1, W2, scale):
    """Gather valid value positions per batch; returns (in_maps, tve)."""
    import ml_dtypes

    query = np.asarray(query, dtype=np.float32)
    value = np.asarray(value, dtype=np.float32)
    mask = np.asarray(mask)
    W1 = np.ascontiguousarray(np.asarray(W1, dtype=np.float32))
    W2 = np.ascontiguousarray(np.asarray(W2, dtype=np.float32))
    scale = np.asarray(scale, dtype=np.float32)

    idxs = [np.nonzero(mask[:, b])[0] for b in range(B)]
    nv_max = max(1, max(len(ix) for ix in idxs))
    tve = min(TV, -(-nv_max // 8) * 8)
    NVC = -(-tve // 128)
    M = KB + 2

    bf16_np = np.dtype(ml_dtypes.bfloat16)
    t, agrid, Fg = _basis_tables()
    ident = np.eye(128, dtype=bf16_np)
    ones1 = np.ones((1, TQL), bf16_np)

    in_maps = []
    for c in range(NCORES):
        b, q0 = c // 2, (c % 2) * TQL
        ix = idxs[b]
        nv = len(ix)
        vg = np.zeros((NVC * 128, D), np.float32)
        vg[:nv] = value[ix, b, :]
        mka = np.zeros((1, tve), bf16_np)
        mka[0, nv:] = NEG_INF

        # host-side a = q W1 and the per-a basis coefficients (interp)
        a = query[q0:q0 + TQL, b, :] @ W1          # [TQL, U]
        ac = np.clip(a, agrid[0], agrid[-1]).ravel()
        F = np.empty((TQL * U, M), np.float32)
        for m in range(M):
            F[:, m] = np.interp(ac, agrid, Fg[:, m])
        F = F.reshape(TQL, U, M)
        # fold the scale vector in; H[m][u,q] = s_u * F[q,u,m]
        H = (scale[None, :, None] * F).transpose(2, 1, 0)  # [M, U, TQL]
        h0 = H[0].sum(axis=0)                      # [TQL] const-term bias
        hpack = np.ascontiguousarray(
            H[1:].transpose(1, 0, 2).reshape(U, (M - 1) * TQL)
        ).astype(bf16_np)
        smalls = np.empty((128, KB + 1), np.float32)
        smalls[:, :KB] = t[None, :]
        smalls[:, KB] = h0

        in_maps.append({
            "vt": np.ascontiguousarray(vg[:tve].T).astype(bf16_np),
            "w2": W2.astype(bf16_np),
            "smalls": smalls,
            "mpack": np.ascontiguousarray(
                np.concatenate([mka, ones1], axis=1)),
            "hpack": hpack,
            "vnp": np.ascontiguousarray(
                vg.reshape(NVC, 128, D).transpose(1, 0, 2)
                .reshape(128, NVC * D)).astype(bf16_np),
            "ident": ident,
        })
    return in_maps, tve


def run(query, value, mask, W1, W2, scale, trace=False):
    from concourse.bass_utils import run_bass_kernel_spmd

    in_maps, tve = prep_in_maps(query, value, mask, W1, W2, scale)
    nc = get_nc(tve)
    res = run_bass_kernel_spmd(nc, in_maps, list(range(NCORES)), trace=trace)
    out = np.empty((TQ, B, D), np.float32)
    for c in range(NCORES):
        b, q0 = c // 2, (c % 2) * TQL
        out[q0:q0 + TQL, b, :] = res.results[c]["out"]
    return out, res


def kernel(query, value, mask, W1, W2, scale):
    out, _ = run(query, value, mask, W1, W2, scale, trace=False)
    return out


# revision 10
# speedup vs baseline: 4.8112x; 1.1474x over previous
"""Bahdanau (additive) attention kernel for Trainium2, 8 NeuronCores.

Full-input contract: kernel(**inputs) takes the unsharded numpy inputs and
returns the full [TQ, B, D] output. Internally shards (batch, query-half)
across 8 cores (B=4 x 2 halves of Tq), runs a Bass/Tile kernel per core via
run_bass_kernel_spmd, and reassembles.

Sparsity: masked value positions contribute exactly 0 to the softmax
(score + -1e9 -> exp underflows to 0), so the host gathers only the valid
value positions per batch (mask is input data), pads to a common TVE
(multiple of 8), and the device program is compiled for that TVE (cached).

Rank-K score decomposition: the additive score
  scores[q,v] = sum_u s_u tanh(a[q,u] + b[v,u]),  a = qW1, b = vW2
is evaluated via a shifted-tanh basis expansion of the bivariate function
  tanh(a+b) ~= f0(a) + f1(a)*b + sum_k fk(a) tanh(b + t_k)
whose per-a coefficients are solved on the host by Gaussian-weighted least
squares (a = wq is host-computed; it is O(Tq*D*U), tiny next to the
O(Tq*Tv*U) score tensor). Folding s_u into the coefficients gives
  scores[q,v] = h0[q] + sum_m H_m[:,q] . TB_m[:,v]
so the device only computes K+1 activation passes over [U, TVE] (the basis
tiles TB_m) and K+2 PE matmuls - instead of Tq tanh passes. h0 is applied
as the per-partition bias of the softmax exp activation (free).

Per-core program (b = batch, 128 local queries, TVE gathered positions):
  warmup matmuls flip the PE clock gate (HAM) during the input DMAs
  wk[u,v] = sum_d W2[d,u] v[v,d]           (PE matmul, bf16 -> PSUM)
  TB_k[u,v] = tanh(wk[u,v] + t_k)          (ACT reads PSUM, bf16 out)
  scores[q,v] = mka[v] + H_1 . wk + sum_k H_k . TB_k   (PE, bf16 PSUM accum)
  e = exp(scores + h0[q]) bank slices, fused row-sum (ACT accum_out)
  ctx[q,d] = (1/ssum[q]) sum_v e[q,v] v[v,d]  (PE transpose + bf16 matmuls;
       DVE scale on the output copy)
"""

import sys

if "/opt/trn_rl_repo" not in sys.path:
    sys.path.insert(0, "/opt/trn_rl_repo")

import numpy as np

TQ, TV, B, D, U = 256, 1024, 4, 128, 128
NCORES = 8
TQL = 128  # local queries per core (Tq=256 split in 2 per batch)
NEG_INF = -1e9

# Basis config: K tanh shifts, placed by offline weighted-LS optimization
# (Nelder-Mead on the (a,b)~N(0,1)^2 rms of the rank-K expansion).
SHIFTS = [-3.703, -2.366, -0.641, -0.08, 0.628, 1.23, 2.189, 3.595]
KB = len(SHIFTS)
AGRID_N = 2001
AGRID_MAX = 5.4
BGRID_N = 601
BGRID_MAX = 6.0

_CACHE = {}


def _bank_pieces(tve):
    """Split [0, tve) into PSUM-bank-aligned matmul slices (<=512 each)."""
    pieces = []
    a = 0
    while a < tve:
        n = min(512, tve - a)
        pieces.append((a, n))
        a += n
    return pieces


def _basis_tables():
    """Least-squares coefficient tables for the shifted-tanh expansion.

    Returns (t, agrid, Fg) with Fg[i, m] the coefficient of basis m
    (m=0 const, m=1 identity, m=2.. tanh(b+t_{m-2})) for a = agrid[i]:
      tanh(a + b) ~= sum_m Fg[i, m] * phi_m(b)   (b ~ N(0,1)-weighted)
    """
    key = "basis"
    if key in _CACHE:
        return _CACHE[key]
    t = np.asarray(SHIFTS, dtype=np.float64)
    bg = np.linspace(-BGRID_MAX, BGRID_MAX, BGRID_N)
    sw = np.sqrt(np.exp(-bg ** 2 / 4))
    cols = [np.ones_like(bg), bg] + [np.tanh(bg + tk) for tk in t]
    A = (np.vstack(cols) * sw).T                  # [nb, M]
    P = np.linalg.pinv(A, rcond=1e-12)            # [M, nb]
    agrid = np.linspace(-AGRID_MAX, AGRID_MAX, AGRID_N)
    Y = np.tanh(agrid[:, None] + bg[None, :]) * sw  # [na, nb]
    Fg = Y @ P.T                                  # [na, M]
    _CACHE[key] = (t, agrid, Fg.astype(np.float64))
    return _CACHE[key]


def _build_nc(tve):
    import concourse.bacc as bacc
    import concourse.mybir as mybir
    import concourse.tile as tile
    from contextlib import ExitStack

    f32 = mybir.dt.float32
    bf16 = mybir.dt.bfloat16
    AFT = mybir.ActivationFunctionType

    nc = bacc.Bacc("TRN2", target_bir_lowering=False, debug=False,
                   num_devices=NCORES)

    NVC = -(-tve // 128)              # ctx chunks (last may be partial)
    pieces = _bank_pieces(tve)
    M = KB + 2                        # const (exp bias) + identity + K tanh

    vt = nc.dram_tensor("vt", [D, tve], bf16, kind="ExternalInput").ap()
    wid = nc.dram_tensor("wid", [D, 2 * U], bf16, kind="ExternalInput").ap()
    smalls = nc.dram_tensor("smalls", [128, KB + 1], f32,
                            kind="ExternalInput").ap()
    mpack = nc.dram_tensor("mpack", [1, tve + TQL], bf16,
                           kind="ExternalInput").ap()
    hpack = nc.dram_tensor("hpack", [U, (M - 1) * TQL], bf16,
                           kind="ExternalInput").ap()
    vnp = nc.dram_tensor("vnp", [128, NVC * D], bf16,
                         kind="ExternalInput").ap()
    out = nc.dram_tensor("out", [TQL, D], f32, kind="ExternalOutput").ap()

    with tile.TileContext(nc) as tc:
        with ExitStack() as ctx:
            consts = ctx.enter_context(tc.tile_pool(name="consts", bufs=1))
            tbp = ctx.enter_context(tc.tile_pool(name="tb", bufs=3))
            smp = ctx.enter_context(tc.tile_pool(name="sm", bufs=1))
            etp = ctx.enter_context(tc.tile_pool(name="et", bufs=2))
            ps1 = ctx.enter_context(tc.tile_pool(name="ps1", bufs=1,
                                                 space="PSUM"))
            pst = ctx.enter_context(tc.tile_pool(name="pst", bufs=2,
                                                 space="PSUM"))

            vt_sb = consts.tile([D, tve], bf16, tag="vt")
            wid_sb = consts.tile([D, 2 * U], bf16, tag="wid")
            w2_sb = wid_sb[:, 0:U]
            id_sb = wid_sb[:, U:2 * U]
            smalls_sb = consts.tile([128, KB + 1], f32, tag="smalls")
            mpack_sb = consts.tile([1, tve + TQL], bf16, tag="mpack")
            mka_sb = mpack_sb[:, 0:tve]
            ones_sb = mpack_sb[:, tve:tve + TQL]
            hpack_sb = consts.tile([U, (M - 1) * TQL], bf16, tag="hpack")
            vnp_sb = consts.tile([128, NVC * D], bf16, tag="vnp")

            # preload the exp/tanh ACT table set during the input DMAs
            warm_in = consts.tile([128, 1], f32, tag="warm_in")
            warm_out = consts.tile([128, 1], f32, tag="warm_out")
            nc.gpsimd.memset(warm_in[:], 0.0)
            nc.scalar.activation(warm_out[:], warm_in[:], AFT.Tanh)

            # flip the PE clock gate (HAM) warm with dummy matmuls while
            # the input DMAs land + complete (kept short: they share the PE
            # queue with wk and would delay it)
            warm_mm = consts.tile([128, 512], bf16, tag="warm_mm")
            nc.gpsimd.memset(warm_mm[:], 0.0)
            warm_ps = pst.tile([128, 512], f32, tag="tp")
            for _ in range(2):
                nc.tensor.matmul(warm_ps[:], lhsT=warm_mm[:, 0:128],
                                 rhs=warm_mm[:], start=True, stop=True)

            # one serialized DMA queue, priority order: vt gates wk -> the
            # whole tanh-basis pipeline; hpack slices are consumed in order
            nc.sync.dma_start(vt_sb[:], vt[:])
            nc.sync.dma_start(wid_sb[:], wid[:])
            nc.sync.dma_start(mpack_sb[:], mpack[:])
            nc.sync.dma_start(hpack_sb[:], hpack[:])
            nc.scalar.dma_start(smalls_sb[:], smalls[:])
            nc.scalar.dma_start(vnp_sb[:], vnp[:])

            # wk = W2.T @ vT -> PSUM (ACT reads PSUM at lower latency)
            wk_ps = ps1.tile([U, tve], f32, tag="wk")
            for a, n in pieces:
                nc.tensor.matmul(wk_ps[:, a:a + n], lhsT=w2_sb[:],
                                 rhs=vt_sb[:, a:a + n])
            # bf16 copy for the identity-basis matmul rhs (off ACT's path)
            wkb_sb = consts.tile([U, tve], bf16, tag="wkb")
            nc.vector.tensor_copy(wkb_sb[:], wk_ps[:])

            scores_ps = ps1.tile([TQL, tve], f32, tag="scores")
            # pad/mask add opens the accumulation-group bookkeeping:
            # scores[q, v] = mka[v]; later matmuls accumulate per-element.
            for a, n in pieces:
                nc.tensor.matmul(scores_ps[:, a:a + n],
                                 lhsT=ones_sb[:], rhs=mka_sb[:, a:a + n],
                                 start=True, stop=True)
            # identity-basis term: scores += H_1 . wk
            for a, n in pieces:
                nc.tensor.matmul(scores_ps[:, a:a + n],
                                 lhsT=hpack_sb[:, 0:TQL],
                                 rhs=wkb_sb[:, a:a + n],
                                 start=False, stop=False,
                                 skip_group_check=True)
            # K shifted-tanh basis tiles; PE consumes each as ACT emits it
            for k in range(KB):
                tb = tbp.tile([U, tve], bf16, tag="tb")
                nc.scalar.activation(tb[:], wk_ps[:], AFT.Tanh,
                                     bias=smalls_sb[:, k:k + 1])
                lw = hpack_sb[:, (k + 1) * TQL:(k + 2) * TQL]
                for a, n in pieces:
                    nc.tensor.matmul(scores_ps[:, a:a + n],
                                     lhsT=lw, rhs=tb[:, a:a + n],
                                     start=False, stop=False,
                                     skip_group_check=True)

            # exp in bank slices with fused row-sums; the const basis term
            # h0[q] rides along as the per-partition activation bias
            exp_sb = smp.tile([TQL, tve], bf16, tag="exp")
            ssums = smp.tile([TQL, len(pieces)], f32, tag="ssums")
            ssum = smp.tile([TQL, 1], f32, tag="ssum")
            rins = smp.tile([TQL, 1], f32, tag="rins")
            for j, (a, n) in enumerate(pieces):
                nc.scalar.activation(exp_sb[:, a:a + n], scores_ps[:, a:a + n],
                                     AFT.Exp, bias=smalls_sb[:, KB:KB + 1],
                                     accum_out=ssums[:, j:j + 1])
            if len(pieces) > 1:
                nc.vector.reduce_sum(ssum[:], ssums[:],
                                     axis=mybir.AxisListType.X)
            else:
                nc.vector.tensor_copy(ssum[:], ssums[:])
            nc.vector.reciprocal(rins[:], ssum[:])

            # ctx = softmax @ v  (transpose exp chunks, accumulate matmuls)
            ctx_ps = ps1.tile([TQL, D], f32, tag="ctx")
            for k in range(NVC):
                n = min(128, tve - k * 128)
                tp = pst.tile([128, 128], bf16, tag="tp")
                nc.tensor.transpose(tp[:n, :],
                                    exp_sb[:, k * 128:k * 128 + n], id_sb[:])
                et = etp.tile([128, 128], bf16, tag="et")
                if k % 2 == 0:
                    nc.vector.tensor_copy(et[:n, :], tp[:n, :])
                else:
                    nc.scalar.copy(et[:n, :], tp[:n, :])
                nc.tensor.matmul(ctx_ps[:], lhsT=et[:n, :],
                                 rhs=vnp_sb[:n, k * D:(k + 1) * D],
                                 start=(k == 0), stop=(k == NVC - 1))

            out_sb = smp.tile([TQL, D], f32, tag="out")
            nc.vector.tensor_scalar_mul(out_sb[:], ctx_ps[:], rins[:])
            nc.sync.dma_start(out[:], out_sb[:])

    nc.compile()
    return nc


def get_nc(tve=TV):
    key = ("nc", tve)
    if key not in _CACHE:
        _CACHE[key] = _build_nc(tve)
    return _CACHE[key]


def prep_in_maps(query, value, mask, W1, W2, scale):
    """Gather valid value positions per batch; returns (in_maps, tve)."""
    import ml_dtypes

    query = np.asarray(query, dtype=np.float32)
    value = np.asarray(value, dtype=np.float32)
    mask = np.asarray(mask)
    W1 = np.ascontiguousarray(np.asarray(W1, dtype=np.float32))
    W2 = np.ascontiguousarray(np.asarray(W2, dtype=np.float32))
    scale = np.asarray(scale, dtype=np.float32)

    idxs = [np.nonzero(mask[:, b])[0] for b in range(B)]
    nv_max = max(1, max(len(ix) for ix in idxs))
    tve = min(TV, -(-nv_max // 8) * 8)
    NVC = -(-tve // 128)
    M = KB + 2

    bf16_np = np.dtype(ml_dtypes.bfloat16)
    t, agrid, Fg = _basis_tables()
    ones1 = np.ones((1, TQL), bf16_np)

    in_maps = []
    for c in range(NCORES):
        b, q0 = c // 2, (c % 2) * TQL
        ix = idxs[b]
        nv = len(ix)
        vg = np.zeros((NVC * 128, D), np.float32)
        vg[:nv] = value[ix, b, :]
        mka = np.zeros((1, tve), bf16_np)
        mka[0, nv:] = NEG_INF

        # host-side a = q W1 and the per-a basis coefficients (interp)
        a = query[q0:q0 + TQL, b, :] @ W1          # [TQL, U]
        ac = np.clip(a, agrid[0], agrid[-1]).ravel()
        F = np.empty((TQL * U, M), np.float32)
        for m in range(M):
            F[:, m] = np.interp(ac, agrid, Fg[:, m])
        F = F.reshape(TQL, U, M)
        # fold the scale vector in; H[m][u,q] = s_u * F[q,u,m]
        H = (scale[None, :, None] * F).transpose(2, 1, 0)  # [M, U, TQL]
        h0 = H[0].sum(axis=0)                      # [TQL] const-term bias
        hpack = np.ascontiguousarray(
            H[1:].transpose(1, 0, 2).reshape(U, (M - 1) * TQL)
        ).astype(bf16_np)
        smalls = np.empty((128, KB + 1), np.float32)
        smalls[:, :KB] = t[None, :]
        smalls[:, KB] = h0

        wid = np.concatenate(
            [W2.astype(bf16_np), np.eye(128, dtype=bf16_np)], axis=1)
        in_maps.append({
            "vt": np.ascontiguousarray(vg[:tve].T).astype(bf16_np),
            "wid": np.ascontiguousarray(wid),
            "smalls": smalls,
            "mpack": np.ascontiguousarray(
                np.concatenate([mka, ones1], axis=1)),
            "hpack": hpack,
            "vnp": np.ascontiguousarray(
                vg.reshape(NVC, 128, D).transpose(1, 0, 2)
                .reshape(128, NVC * D)).astype(bf16_np),
        })
    return in_maps, tve


def run(query, value, mask, W1, W2, scale, trace=False):
    from concourse.bass_utils import run_bass_kernel_spmd

    in_maps, tve = prep_in_maps(query, value, mask, W1, W2, scale)
    nc = get_nc(tve)
    res = run_bass_kernel_spmd(nc, in_maps, list(range(NCORES)), trace=trace)
    out = np.empty((TQ, B, D), np.float32)
    for c in range(NCORES):
        b, q0 = c // 2, (c % 2) * TQL
        out[q0:q0 + TQL, b, :] = res.results[c]["out"]
    return out, res


def kernel(query, value, mask, W1, W2, scale):
    out, _ = run(query, value, mask, W1, W2, scale, trace=False)
    return out
